# revision 44
# baseline (speedup 1.0000x reference)
"""EpsSupInfoNCE loss on 8 Trainium2 NeuronCores — symmetry-dedup version.

logits = (E@E.T)/temp is SYMMETRIC, so each off-diagonal exp is computed
ONCE device-wide and feeds BOTH sums it belongs to: S for its column via
the ACT fused per-instruction accumulator, and S for its row via a
ones-vector matmul over P on the (underutilized) tensor engine.

Layout (host sorts all rows/cols by label first):
- core a owns cols [1024a, 1024(a+1)); 8 col-tiles of 128.
- window = rows [1024a, 1024a+2048) mod B  (own block + next block),
  MASKED via one-hot matmul (-4.5 in dot units -> -C in logit units).
  Own-block rows feed column-accum only (each within-block pair appears
  twice, once per mirror entry -> once per S side). Next-block rows feed
  column-accum AND a ones-matmul row-sum.
- main = sliding prefix of a circular band band[x] = 1024a+2048+x,
  x < 3072: col-tile k computes band[0:(17+k)*128]; rows [0:(16+k)*128]
  feed accum+ones; the last 128 rows (= tile u+32 for col-tile u) feed
  accum ONLY — its mirror tile does the same, covering diff-32 pairs
  once per side. All other cross-block tile pairs {u, v} are covered
  exactly once by the circular tournament v in {u+1..u+31}.
- numerator: P_win[:, 0:1280] shipped raw to DRAM; host recovers
  l = ln(P)+C at same-label entries (upper triangle, row>col) and
  computes both ordered ce terms from the single symmetric value.
- row-sums: ones[128,128] (fp32r) lhsT replicates each 512-chunk's sums
  across partitions into a rotating 2-bank PSUM tile; one DVE add per
  block folds them into an SBUF accumulator (PE fp32r PSUM writes must
  start at partition 0, so strip-packing is not available). Ones blocks
  are emitted with depth-2 deferral so the in-order PE queue never
  stalls on ACT, keeping the HAM clock up. PSUM: 2x2-bank groups +
  2x2-bank ones = 8 banks.
"""
import numpy as np
import ml_dtypes
from contextlib import ExitStack

import concourse.bacc as bacc
import concourse.tile as tile
from concourse import mybir
from concourse.bass_utils import run_bass_kernel_spmd

B, D = 8192, 128
NCLS = 100
NCORES = 8
COLS = B // NCORES            # 1024 columns per core
NCT = COLS // 128             # 8 col-tiles per core
WIN = 2048                    # window rows per core (blocks a, a+1)
BAND = 3072                   # circular main band rows per core
LNW = 1280                    # shipped window rows (numerator span)
G1 = 1024                     # PSUM group width (2 banks; ones need 3,
                              # PE PSUM col base is limited to {0,32,64})
FIRST = 512                   # m0's tiny first chunk (early ACT start)

TEMP = 0.07
EPS = 0.25
SCALE = float(np.float32(1.0) / np.float32(TEMP))   # exp scale (fp32 value)
MASKVAL = -4.5                                      # bf16-exact additive mask
C_USED = 4.5 * SCALE                                # mask size in logit units

# program order: window k ships 640KB of P, so keep the last mains after
# the last window to hide the final ship under trailing ACT work.
ORDER = ["m0", "m1", "m2", "w0", "m3", "w1", "m4", "w2", "m5", "w3",
         "w4", "w5", "w6", "w7", "m6", "m7"]

_cache = {}


def _patch_act_tables():
    """Steer Exp onto a stable table set (baseline's patch; harmless now
    that only Exp is used)."""
    import concourse.hw_specs as hw_specs
    from concourse import mybir as _mb
    if getattr(bacc, "_act_tables_patched", False):
        return
    orig = hw_specs.get_activation_tables

    def steer(arch):
        t = orig(arch)
        exp, ln = (_mb.ActivationFunctionType.Exp, _mb.ActivationFunctionType.Ln)
        if "natural_log_exp_and_others" not in t:
            return t
        return {k: (fns if k == "natural_log_exp_and_others"
                    else fns - {exp, ln}) for k, fns in t.items()}

    bacc.get_activation_tables = steer
    bacc._act_tables_patched = True


def _slots():
    """Deterministic accum-slot layout shared by _build and _combine.
    Returns (slot_of[step_name] -> list of slot ids, NS)."""
    slot_of = {}
    n = 0
    for step in ORDER:
        if step[0] == "m":
            nslots = 3 + (1 if step == "m0" else 0)
        else:
            nslots = 2
        slot_of[step] = list(range(n, n + nslots))
        n += nslots
    return slot_of, n


def _build():
    dt = mybir.dt
    _patch_act_tables()
    nc = bacc.Bacc("TRN2", target_bir_lowering=False, debug=False,
                   num_devices=NCORES)
    et_own = nc.dram_tensor("et_own", [D, COLS], dt.bfloat16,
                            kind="ExternalInput").ap()
    et_win = nc.dram_tensor("et_win", [D, WIN], dt.bfloat16,
                            kind="ExternalInput").ap()
    et_band = nc.dram_tensor("et_band", [D, BAND], dt.bfloat16,
                             kind="ExternalInput").ap()
    oh_win = nc.dram_tensor("oh_win", [NCLS, WIN], dt.bfloat16,
                            kind="ExternalInput").ap()
    ohn_own = nc.dram_tensor("ohn_own", [NCLS, COLS], dt.bfloat16,
                             kind="ExternalInput").ap()
    slot_of, NS = _slots()
    out = nc.dram_tensor("out", [128, NS], dt.float32,
                         kind="ExternalOutput").ap()
    # P is produced as float32r (PE consumes it as fp32r moving data);
    # same bits as f32 on the host side.
    pout = nc.dram_tensor("pout", [128, NCT * LNW], dt.float32r,
                          kind="ExternalOutput").ap()
    ones_out = nc.dram_tensor("ones_out", [1, 5120], dt.float32,
                              kind="ExternalOutput").ap()

    with tile.TileContext(nc) as tc:
        with ExitStack() as ctx:
            const_pool = ctx.enter_context(tc.tile_pool(name="consts", bufs=1))
            p_pool = ctx.enter_context(tc.tile_pool(name="pwin", bufs=3))
            d_pool = ctx.enter_context(tc.tile_pool(name="pmain", bufs=5))
            ps_pool = ctx.enter_context(
                tc.tile_pool(name="psum", bufs=2, space="PSUM"))
            po_pool = ctx.enter_context(
                tc.tile_pool(name="psones", bufs=2, space="PSUM"))

            t_et_own = const_pool.tile([D, COLS], dt.bfloat16)
            t_et_win = const_pool.tile([D, WIN], dt.bfloat16)
            t_et_band = const_pool.tile([D, BAND], dt.bfloat16)
            t_oh_win = const_pool.tile([NCLS, WIN], dt.bfloat16)
            t_ohn_own = const_pool.tile([NCLS, COLS], dt.bfloat16)
            # Inputs: head on sync (first exp waits ~160KB), bulk split
            # so the 5.2MB of P ships own the sync hw queue afterwards.
            # oh_win[1536:] is never read (mask is trimmed past 1536).
            nc.sync.dma_start(t_et_own[:, 0:128], et_own[:, 0:128])
            nc.sync.dma_start(t_et_band[:, 0:FIRST], et_band[:, 0:FIRST])
            nc.sync.dma_start(t_et_band[:, FIRST:G1], et_band[:, FIRST:G1])
            nc.sync.dma_start(t_et_band[:, G1:2432], et_band[:, G1:2432])
            nc.gpsimd.dma_start(t_et_win[:, G1:], et_win[:, G1:])
            nc.sync.dma_start(t_et_win[:, 0:G1], et_win[:, 0:G1])
            nc.gpsimd.dma_start(t_oh_win[:, 0:1536], oh_win[:, 0:1536])
            nc.gpsimd.dma_start(t_ohn_own[:, 0:128], ohn_own[:, 0:128])
            nc.sync.dma_start(t_et_own[:, 128:], et_own[:, 128:])
            nc.gpsimd.dma_start(t_et_band[:, 2432:], et_band[:, 2432:])
            nc.gpsimd.dma_start(t_ohn_own[:, 128:], ohn_own[:, 128:])

            ones_f = const_pool.tile([128, 128], dt.float32)
            nc.gpsimd.memset(ones_f[:], 1.0)
            ones_t = const_pool.tile([128, 128], dt.float32r)
            nc.scalar.copy(ones_t[:], ones_f[:])   # legal fp32r producer
            ones_r = ones_t[:]

            s_part = const_pool.tile([128, NS], dt.float32)
            # Row-sums: ones[128,128] lhsT replicates each chunk's sums
            # across all 128 partitions (PE PSUM writes must start at
            # partition 0 for fp32r). Each step's chunks land in a fresh
            # 2-bank PSUM tile; one DVE add folds them into acc_sb.
            # Slot s of acc_sb covers free [512s, 512s+512).
            acc_sb = const_pool.tile([128, 5120], dt.float32)
            nc.gpsimd.memset(acc_sb[:], 0.0)

            # Deferred ones emission: the PE queue is in-order, so a ones
            # matmul right after its group's EXP stalls the PE on ACT
            # every group (keeping the HAM clock throttled). Queue each
            # block and emit it one group later, when its P is ready.
            pending_ones = []

            depth = {"v": 3}

            def flush_ones(keep=None):
                # depth-2 deferral: EXP(i)+readout outlast group i+1's
                # matmuls, so emit ones(i) during group i+2. The last two
                # m-steps drain eagerly (ACT paces the end, PE has slack)
                # so the post-loop tail chain is one block, not four.
                if len(pending_ones) <= (depth["v"] if keep is None else keep):
                    return
                pairs, acc_lo, acc_w = pending_ones.pop(0)
                po = po_pool.tile([128, G1], dt.float32, tag="po")
                off = 0
                for rhs, w in pairs:
                    nc.tensor.matmul(po[:, off:off + w], ones_r, rhs,
                                     start=True, stop=True)
                    off += w
                assert off == acc_w
                nc.vector.tensor_add(
                    acc_sb[:, acc_lo:acc_lo + acc_w],
                    acc_sb[:, acc_lo:acc_lo + acc_w], po[:, 0:acc_w])

            def ones_block(pairs, acc_lo, acc_w):
                pending_ones.append((pairs, acc_lo, acc_w))

            def group(lhs_et, rhs_t, r0, w, P, p_off, slot, lhs_oh=None,
                      oh_t=None, split=False, mw=None):
                """One PSUM group: logit (+ mask over [0:mw]) matmuls,
                then EXP with fused accum into s_part[slot]."""
                ps = ps_pool.tile([128, G1], dt.float32, tag="ps")
                mw = w if mw is None else mw
                off = 0
                while off < w:
                    cw = min(512, w - off)
                    nc.tensor.matmul(ps[:, off:off + cw], lhs_et,
                                     rhs_t[:, r0 + off:r0 + off + cw],
                                     start=True,
                                     stop=(lhs_oh is None or off >= mw))
                    off += cw
                flush_ones()       # previous group's P is ready by now
                if lhs_oh is not None:
                    off = 0
                    while off < mw:
                        cw = min(512, mw - off)
                        nc.tensor.matmul(ps[:, off:off + cw], lhs_oh,
                                         oh_t[:, r0 + off:r0 + off + cw],
                                         start=False, stop=True)
                        off += cw
                if split:
                    nc.scalar.activation(
                        P[:, p_off:p_off + FIRST], ps[:, 0:FIRST],
                        mybir.ActivationFunctionType.Exp,
                        scale=SCALE, accum_out=s_part[:, slot:slot + 1])
                    nc.scalar.activation(
                        P[:, p_off + FIRST:p_off + w], ps[:, FIRST:w],
                        mybir.ActivationFunctionType.Exp,
                        scale=SCALE, accum_out=s_part[:, slot + 1:slot + 2])
                else:
                    nc.scalar.activation(
                        P[:, p_off:p_off + w], ps[:, 0:w],
                        mybir.ActivationFunctionType.Exp,
                        scale=SCALE, accum_out=s_part[:, slot:slot + 1])

            def w_step(k):
                lhs_et = t_et_own[:, k * 128:(k + 1) * 128]
                lhs_oh = t_ohn_own[:, k * 128:(k + 1) * 128]
                sl = slot_of[f"w{k}"]
                own_w = (8 - k) * 128
                P = p_pool.tile([128, WIN], dt.float32r, tag="P")
                # W1: own rows [128k:1024] only (triangular: tiles below
                # the diagonal are the mirror of earlier col-tiles' work).
                # Diagonal tile (P[:,0:128]) has both orientations ->
                # accum only; rows past it also feed ones (their mirror
                # is never computed).
                group(lhs_et, t_et_win, 128 * k, own_w, P, 0, sl[0],
                      lhs_oh=lhs_oh, oh_t=t_oh_win)
                if k < 7:
                    ow = (7 - k) * 128
                    pairs = [(P[:, 128:128 + min(512, ow)], min(512, ow))]
                    if ow > 512:
                        pairs.append((P[:, 640:128 + ow], ow - 512))
                    ones_block(pairs, 4096 + 128 * (k + 1), ow)
                # W2: FWD rows [1024:2048] — accum + ones; same-label
                # rows end < 1024a+1280, so mask only [1024:1536].
                group(lhs_et, t_et_win, G1, G1, P, own_w, sl[1],
                      lhs_oh=lhs_oh, oh_t=t_oh_win, mw=512)
                ones_block([(P[:, own_w:own_w + 512], 512),
                            (P[:, own_w + 512:own_w + 1024], 512)],
                           3072, 1024)
                # ship numerator rows (sync = hardware DGE queue)
                sw = min(LNW, own_w + G1)
                nc.sync.dma_start(pout[:, k * LNW:k * LNW + sw],
                                  P[:, 0:sw])

            def m_step(k):
                lhs_et = t_et_own[:, k * 128:(k + 1) * 128]
                sl = slot_of[f"m{k}"]
                n_main = (17 + k) * 128
                n_ones = (16 + k) * 128
                # groups over band rows [0:1024), [1024:2048), [2048:n_main)
                Pa = d_pool.tile([128, G1], dt.float32r, tag="Pm")
                group(lhs_et, t_et_band, 0, G1, Pa, 0, sl[0],
                      split=(k == 0))
                si = 2 if k == 0 else 1
                ones_block([(Pa[:, 0:512], 512), (Pa[:, 512:1024], 512)],
                           0, 1024)
                Pb = d_pool.tile([128, G1], dt.float32r, tag="Pm")
                group(lhs_et, t_et_band, G1, G1, Pb, 0, sl[si])
                ones_block([(Pb[:, 0:512], 512), (Pb[:, 512:1024], 512)],
                           1024, 1024)
                wd = n_main - 2048
                Pc = d_pool.tile([128, G1], dt.float32r, tag="Pm")
                group(lhs_et, t_et_band, 2048, wd, Pc, 0, sl[si + 1])
                # ones over band rows [2048:n_ones] (width 128k)
                w3 = n_ones - 2048
                if w3 > 0:
                    pairs = [(Pc[:, 0:min(512, w3)], min(512, w3))]
                    if w3 > 512:
                        pairs.append((Pc[:, 512:w3], w3 - 512))
                    ones_block(pairs, 2048, w3)

            for step in ORDER:
                if step == "m6":
                    depth["v"] = 0
                (m_step if step[0] == "m" else w_step)(int(step[1]))
            while pending_ones:
                flush_ones(keep=0)

            nc.sync.dma_start(out[:], s_part[:])
            nc.gpsimd.dma_start(ones_out[:], acc_sb[0:1, :])
    nc.compile()
    return nc


def _get_nc():
    if "nc" not in _cache:
        _cache["nc"] = _build()
    return _cache["nc"]


def _prepare(embeds, labels):
    embeds = np.ascontiguousarray(np.asarray(embeds, dtype=np.float32))
    labels_i = np.asarray(labels).astype(np.int64)
    assert embeds.shape == (B, D)

    perm = np.argsort(labels_i, kind="stable")
    lab = labels_i[perm]
    emb = embeds[perm]

    et = np.ascontiguousarray(emb.T).astype(ml_dtypes.bfloat16)   # [D, B]
    oh = np.zeros((NCLS, B), dtype=ml_dtypes.bfloat16)
    oh[lab, np.arange(B)] = ml_dtypes.bfloat16(1.0)
    ohn = (oh.astype(np.float32) * np.float32(MASKVAL)).astype(
        ml_dtypes.bfloat16)

    in_maps = []
    for c in range(NCORES):
        lo = c * COLS
        win = (lo + np.arange(WIN)) % B
        band = (lo + WIN + np.arange(BAND)) % B
        r_hi = np.searchsorted(lab, lab[lo + COLS - 1], side="right")
        assert r_hi - lo <= LNW, f"window overflow: {r_hi - lo}"
        in_maps.append({
            "et_own": np.ascontiguousarray(et[:, lo:lo + COLS]),
            "et_win": np.ascontiguousarray(et[:, win]),
            "et_band": np.ascontiguousarray(et[:, band]),
            "oh_win": np.ascontiguousarray(oh[:, win]),
            "ohn_own": np.ascontiguousarray(ohn[:, lo:lo + COLS]),
        })
    return in_maps, lab


def _combine(results, lab):
    slot_of, NS = _slots()
    S = np.zeros(B, dtype=np.float64)
    for a in range(NCORES):
        o = results[a]["out"]
        oo = results[a]["ones_out"].astype(np.float64).reshape(-1)
        sp = o[:, 0:NS].astype(np.float64)
        # column accums: sum each col-tile's slots
        for k in range(NCT):
            cols = a * COLS + k * 128 + np.arange(128)
            acc = np.zeros(128)
            for st in (f"m{k}", f"w{k}"):
                for s in slot_of[st]:
                    acc += sp[:, s]
            S[cols] += acc
        # ones row-sums
        win = (a * COLS + np.arange(WIN)) % B
        band = (a * COLS + WIN + np.arange(BAND)) % B
        S[band] += oo[0:BAND]                   # main slots 0..5
        S[win[1024:2048]] += oo[3072:4096]      # window FWD slots 6,7
        S[win[0:1024]] += oo[4096:5120]         # own-block triangular ones

    counts = np.bincount(lab, minlength=NCLS)
    count_j = counts[lab].astype(np.float64) - 1.0
    logS = np.log(S)

    total = 0.0
    for a in range(NCORES):
        pw = results[a]["pout"].reshape(128, NCT, LNW)
        for k in range(NCT):
            # shipped rows: own tiles k..7, then the FWD block
            own_w = (8 - k) * 128
            wk = min(LNW, own_w + 1024)
            win = np.concatenate([
                a * COLS + k * 128 + np.arange(own_w),
                (a + 1) * COLS + np.arange(wk - own_w)]) % B
            lab_win = lab[win]
            cols = a * COLS + k * 128 + np.arange(128)
            m = (lab[cols][:, None] == lab_win[None, :]) \
                & (win[None, :] > cols[:, None])
            pj, rj = np.nonzero(m)
            i_idx = cols[pj]
            j_idx = win[rj]
            l = np.log(pw[pj, k, rj].astype(np.float64)) + C_USED
            ce_ij = np.logaddexp(-EPS, logS[j_idx] - l)
            ce_ji = np.logaddexp(-EPS, logS[i_idx] - l)
            total += (ce_ij / count_j[j_idx]).sum() \
                + (ce_ji / count_j[i_idx]).sum()

    loss = total / B
    return np.asarray(loss, dtype=np.float32)


def kernel(embeds, labels):
    in_maps, lab = _prepare(embeds, labels)
    nc = _get_nc()
    res = run_bass_kernel_spmd(nc, in_maps, list(range(NCORES)))
    return _combine(res.results, lab)
